# revision 1
# baseline (speedup 1.0000x reference)
"""Trainium2 Bass kernel for nn_EnhancedStateEncoder (6-layer dense transformer).

Strategy: data-parallel over batch across 8 NeuronCores (2 batches/core).
 - Embedding + sinusoidal pos-emb folded on host (cheap gather).
 - Alibi bias [H,S,S] precomputed host-side (input-independent, cached),
   stored transposed+prescaled as bf16, resident in SBUF, and added to
   Q@K^T PSUM via identity-matmul accumulation on the TensorEngine.
 - Attention computed in S^T layout [j(part), i(free)] so exp'd scores feed
   the PV matmul directly as the moving operand; a ones-column appended to V
   yields softmax denominators for free (M=33 stationary).
 - exp on ScalarE with the 1/sqrt(d) scale fused into the activation.
 - LayerNorm via bn_stats/bn_aggr; rsqrt computed as exp(-0.5*ln(var+eps))
   to stay inside the natural_log_exp activation-table set.
 - LN2's affine is folded into the MLP's first matmul on the host.
"""

import math
import os
from contextlib import ExitStack

import numpy as np
import ml_dtypes

import concourse.bass as bass
import concourse.mybir as mybir
import concourse.tile as tile
from concourse.bass_utils import run_bass_kernel_spmd
from concourse.masks import make_identity

F32 = mybir.dt.float32
BF16 = mybir.dt.bfloat16

B, S, D, H, HD, L, H2 = 16, 1024, 256, 8, 32, 6, 1024
NC = 8            # cores
BL = B // NC      # batches per core = 2
T = BL * S        # tokens per core = 2048
NCH = T // 128    # 128-token chunks per core = 16
SCALE = 1.0 / math.sqrt(HD)
LN_EPS = 1e-5
GRID = 32

_cache = {}


def _alibi_biasT():
    """biasT[p, h, jc, i] = bias[h, i, jc*128+p] / SCALE, bf16."""
    if "biasT" in _cache:
        return _cache["biasT"]
    xs, ys = np.meshgrid(np.arange(GRID), np.arange(GRID), indexing="ij")
    xf = xs.reshape(-1).astype(np.float32)
    yf = ys.reshape(-1).astype(np.float32)
    dist = np.abs(xf[:, None] - xf[None, :]) + np.abs(yf[:, None] - yf[None, :])
    i = np.arange(H, dtype=np.float32)
    sl = -(2.0 ** (-(1.0 + i)))
    sr = -(2.0 ** (-(0.5 + i)))
    out = np.empty((128, H, S // 128, S), dtype=ml_dtypes.bfloat16)
    triu = np.triu(np.ones((S, S), np.bool_))  # j >= i
    for h in range(H):
        b = np.where(triu, sr[h] * dist, sl[h] * dist) / SCALE  # [i, j]
        bT = np.ascontiguousarray(b.T)  # [j, i]
        out[:, h] = bT.reshape(S // 128, 128, S).transpose(1, 0, 2)
    _cache["biasT"] = out
    return out


def _pos_table():
    if "pos" in _cache:
        return _cache["pos"]
    inv_freq = 1.0 / (10000.0 ** (np.arange(0, D, 2, dtype=np.float32) / D))
    t = np.arange(S, dtype=np.float32)
    sinusoid = t[:, None] * inv_freq[None, :]
    _cache["pos"] = np.concatenate(
        [np.sin(sinusoid), np.cos(sinusoid)], axis=-1
    ).astype(np.float32)
    return _cache["pos"]


def _build_bass():
    if "nc" in _cache:
        return _cache["nc"]
    nc = bass.Bass()
    io = {}
    io["x0"] = nc.dram_tensor("x0", [128, NCH, D], F32, kind="ExternalInput")
    io["biasT"] = nc.dram_tensor("biasT", [128, H, S // 128, S], BF16, kind="ExternalInput")
    io["w1h"] = nc.dram_tensor("w1h", [L, 128, D // 128, H2], F32, kind="ExternalInput")
    io["b1h"] = nc.dram_tensor("b1h", [L, 128, H2 // 128], F32, kind="ExternalInput")
    io["w2h"] = nc.dram_tensor("w2h", [L, 128, H2 // 128, D], BF16, kind="ExternalInput")
    io["b2h"] = nc.dram_tensor("b2h", [L, 1, D], F32, kind="ExternalInput")
    io["ln1w"] = nc.dram_tensor("ln1w", [L, 128, D], F32, kind="ExternalInput")
    io["ln1b"] = nc.dram_tensor("ln1b", [L, 128, D], F32, kind="ExternalInput")
    io["lnfw"] = nc.dram_tensor("lnfw", [128, D], F32, kind="ExternalInput")
    io["lnfb"] = nc.dram_tensor("lnfb", [128, D], F32, kind="ExternalInput")
    y = nc.dram_tensor("y", [128, NCH, D], F32, kind="ExternalOutput")

    with tile.TileContext(nc) as tc, ExitStack() as ctx:
        _emit(ctx, tc, io, y)

    _split_multi_waits(nc)
    _cache["nc"] = nc
    return nc


def _split_multi_waits(nc):
    """walrus codegen on this image only supports ONE sync-wait per TPB
    engine-instruction descriptor. Move excess waits onto sequencer NoOps
    inserted immediately before the instruction (same engine queue)."""
    nsplit = 0
    skip = ("InstNoOp", "InstEventSemaphore")
    for func in nc.m.functions:
        for bb in func.blocks:
            insts = list(bb.instructions)
            out = []
            for inst in insts:
                si = inst.sync_info
                if (si is not None and si.on_wait and len(si.on_wait) > 1
                        and type(inst).__name__ not in skip):
                    for w in list(si.on_wait[:-1]):
                        nop = mybir.InstNoOp(
                            name=f"WSPLIT-{nsplit}", ins=[], outs=[])
                        nop.engine = inst.engine
                        nop.sync_info = mybir.SyncInfo(
                            on_wait=[w], on_update=[])
                        out.append(nop)
                        nsplit += 1
                    si.on_wait = [si.on_wait[-1]]
                out.append(inst)
            if nsplit:
                bb.instructions = out
    return nsplit


def _bcast_row(dram_ap, n):
    """AP that reads a [n]-shaped dram row broadcast across 128 partitions."""
    return bass.AP(
        tensor=dram_ap.tensor,
        offset=dram_ap.offset,
        ap=[[0, 128]] + dram_ap.ap,
    )


def _emit(ctx, tc, io, y):
    nc = tc.nc
    singles = ctx.enter_context(tc.tile_pool(name="singles", bufs=1))
    lnp = ctx.enter_context(tc.tile_pool(name="lnp", bufs=1))
    wp = ctx.enter_context(tc.tile_pool(name="wp", bufs=1))
    xp = ctx.enter_context(tc.tile_pool(name="xp", bufs=2))
    sp = ctx.enter_context(tc.tile_pool(name="sp", bufs=4))
    ep = ctx.enter_context(tc.tile_pool(name="ep", bufs=2))
    otp = ctx.enter_context(tc.tile_pool(name="otp", bufs=1))
    # PSUM pools: small(1 bank)x2 + big(2 banks)x2 + pv(2 banks)x1 = 8 banks
    ps_small = ctx.enter_context(tc.tile_pool(name="ps_small", bufs=2, space="PSUM"))
    ps_big = ctx.enter_context(tc.tile_pool(name="ps_big", bufs=2, space="PSUM"))
    ps_pv = ctx.enter_context(tc.tile_pool(name="ps_pv", bufs=1, space="PSUM"))

    # ---- resident tensors ----
    x_sb = singles.tile([128, NCH, D], F32)
    nc.sync.dma_start(out=x_sb, in_=io["x0"][:])
    bias_sb = singles.tile([128, H, S // 128, S], BF16)
    nc.sync.dma_start(out=bias_sb, in_=io["biasT"][:])
    id_f32 = singles.tile([128, 128], F32)
    make_identity(nc, id_f32)
    id_bf16 = singles.tile([128, 128], BF16)
    nc.gpsimd.tensor_copy(out=id_bf16, in_=id_f32)
    ones_col = singles.tile([1, 128], F32)
    nc.vector.memset(ones_col, 1.0)
    eps_t = singles.tile([128, 1], F32)
    nc.vector.memset(eps_t, LN_EPS)
    absorb_scratch = singles.tile([128, 16], F32)
    absorb_n = [0]

    def absorb(ap):
        # DVE wait absorber: DVE-struct instructions support only one sync
        # wait on this codegen, so soak the DMA-completion wait into a copy.
        # Disjoint dest columns so absorbers carry no WAW dep on each other.
        k = absorb_n[0] % 16
        absorb_n[0] += 1
        nc.vector.tensor_copy(out=absorb_scratch[:, k:k + 1],
                              in_=ap[0:128, 0:1])
    v_aug = singles.tile([128, NCH, H, 34], BF16)
    nc.vector.memset(v_aug, 1.0)
    # xnT: [half][128, T] transposed layernormed activations
    xnT = []
    for i in range(2):
        xnT_half = singles.tile([128, T], F32, tag=f"xnT{i}")
        xnT.append(xnT_half)
    hT = singles.tile([128, 8, 512], BF16)

    def layer_norm_chunks(x_get, affine, out_cb, inplace_into=None):
        """x_get(c)->AP [128, D]; affine: None or (w_sb, b_sb);
        out_cb(c, xn_ap) consumes normalized chunk."""
        mv_all = sp.tile([128, NCH, 2], F32, tag="mv")
        rs_all = sp.tile([128, NCH], F32, tag="rs")
        for c in range(NCH):
            st = sp.tile([128, 6], F32, tag="st")
            nc.vector.bn_stats(out=st, in_=x_get(c))
            nc.vector.bn_aggr(out=mv_all[:, c, :], in_=st)
        nc.scalar.activation(
            out=rs_all, in_=mv_all[:, :, 1],
            func=mybir.ActivationFunctionType.Ln, bias=eps_t, scale=1.0,
        )
        nc.scalar.activation(
            out=rs_all, in_=rs_all,
            func=mybir.ActivationFunctionType.Exp, scale=-0.5,
        )
        for c in range(NCH):
            if inplace_into is not None:
                xn = inplace_into(c)
            else:
                xn = xp.tile([128, D], F32, tag="xn")
            nc.vector.tensor_scalar(
                out=xn, in0=x_get(c),
                scalar1=mv_all[:, c, 0:1], scalar2=rs_all[:, c:c + 1],
                op0=mybir.AluOpType.subtract, op1=mybir.AluOpType.mult,
            )
            if affine is not None:
                w_sb, b_sb = affine
                nc.vector.tensor_mul(out=xn, in0=xn, in1=w_sb)
                nc.vector.tensor_add(out=xn, in0=xn, in1=b_sb)
            out_cb(c, xn)

    def transpose_to(xn, c):
        for half in range(2):
            pt = ps_small.tile([128, 128], F32, tag="small")
            nc.tensor.transpose(pt, xn[:, half * 128:(half + 1) * 128], id_f32)
            nc.vector.tensor_copy(
                out=xnT[half][:, c * 128:(c + 1) * 128], in_=pt
            )

    for l in range(L):
        # per-layer params
        ln1w_sb = lnp.tile([128, D], F32, tag="ln1w")
        nc.sync.dma_start(out=ln1w_sb, in_=io["ln1w"][l])
        absorb(ln1w_sb)
        ln1b_sb = lnp.tile([128, D], F32, tag="ln1b")
        nc.sync.dma_start(out=ln1b_sb, in_=io["ln1b"][l])
        absorb(ln1b_sb)
        w1_sb = wp.tile([128, D // 128, H2], F32, tag="w1")
        nc.sync.dma_start(out=w1_sb, in_=io["w1h"][l])
        b1_sb = wp.tile([128, H2 // 128], F32, tag="b1")
        nc.sync.dma_start(out=b1_sb, in_=io["b1h"][l])
        w2_sb = wp.tile([128, H2 // 128, D], BF16, tag="w2")
        nc.sync.dma_start(out=w2_sb, in_=io["w2h"][l])
        b2_sb = wp.tile([1, D], F32, tag="b2")
        nc.sync.dma_start(out=b2_sb, in_=io["b2h"][l])

        # ---- phase A: LN1 -> xn, v_aug, xnT ----
        def phase_a(c, xn, l=l):
            nc.vector.tensor_copy(
                out=v_aug[:, c, :, 0:HD],
                in_=xn.rearrange("p (h d) -> p h d", h=H),
            )
            transpose_to(xn, c)

        layer_norm_chunks(
            lambda c: x_sb[:, c, :], (ln1w_sb, ln1b_sb), phase_a
        )

        # ---- phase B: attention per (batch, head) ----
        for b in range(BL):
            for h in range(H):
                xnT_h = xnT[h // 4]
                hp = (h % 4) * HD
                po = ps_pv.tile([33, S], F32, tag="pv")
                for jc in range(S // 128):
                    ps = ps_big.tile([128, S], F32, tag="big")
                    ktile = xnT_h[hp:hp + HD,
                                  b * S + jc * 128: b * S + (jc + 1) * 128]
                    for it in range(2):
                        qtile = xnT_h[hp:hp + HD,
                                      b * S + it * 512: b * S + (it + 1) * 512]
                        nc.tensor.matmul(
                            ps[:, it * 512:(it + 1) * 512],
                            lhsT=ktile, rhs=qtile, start=True, stop=False,
                            tile_position=(hp, 0),
                        )
                    for it in range(2):
                        nc.tensor.matmul(
                            ps[:, it * 512:(it + 1) * 512],
                            lhsT=id_bf16,
                            rhs=bias_sb[:, h, jc, it * 512:(it + 1) * 512],
                            start=False, stop=True,
                        )
                    et = ep.tile([128, S], BF16, tag="et")
                    nc.scalar.activation(
                        out=et, in_=ps,
                        func=mybir.ActivationFunctionType.Exp, scale=SCALE,
                    )
                    for it in range(2):
                        nc.tensor.matmul(
                            po[:, it * 512:(it + 1) * 512],
                            lhsT=v_aug[:, b * 8 + jc, h, 0:33],
                            rhs=et[:, it * 512:(it + 1) * 512],
                            start=(jc == 0), stop=(jc == S // 128 - 1),
                        )
                ot = otp.tile([33, S], F32, tag="ot")
                nc.vector.tensor_copy(out=ot, in_=po)
                for ic in range(S // 128):
                    ptt = ps_small.tile([128, 33], F32, tag="small")
                    nc.tensor.transpose(
                        ptt, ot[:, ic * 128:(ic + 1) * 128], id_f32[0:33, 0:33]
                    )
                    rt = sp.tile([128, 1], F32, tag="rt")
                    nc.vector.reciprocal(out=rt, in_=ptt[:, 32:33])
                    c = b * 8 + ic
                    xs = x_sb[:, c, h * HD:(h + 1) * HD]
                    nc.vector.scalar_tensor_tensor(
                        out=xs, in0=ptt[:, 0:HD], scalar=rt, in1=xs,
                        op0=mybir.AluOpType.mult, op1=mybir.AluOpType.add,
                    )

        # ---- phase C: LN2 (affine folded into w1) -> xnT ----
        layer_norm_chunks(lambda c: x_sb[:, c, :], None,
                          lambda c, xn: transpose_to(xn, c))

        # ---- phase D: MLP ----
        for tt in range(T // 512):
            for hbp in range(4):  # pairs of h2-blocks
                pm = ps_big.tile([128, S], F32, tag="big")
                for sub in range(2):
                    hb = hbp * 2 + sub
                    for k in range(D // 128):
                        nc.tensor.matmul(
                            pm[:, sub * 512:(sub + 1) * 512],
                            lhsT=w1_sb[:, k, hb * 128:(hb + 1) * 128],
                            rhs=xnT[k][:, tt * 512:(tt + 1) * 512],
                            start=(k == 0), stop=(k == D // 128 - 1),
                        )
                for sub in range(2):
                    hb = hbp * 2 + sub
                    nc.scalar.activation(
                        out=hT[:, hb, :],
                        in_=pm[:, sub * 512:(sub + 1) * 512],
                        func=mybir.ActivationFunctionType.Gelu,
                        bias=b1_sb[:, hb:hb + 1],
                    )
            for t2 in range(4):
                pm2 = ps_small.tile([128, D], F32, tag="small")
                for hb in range(H2 // 128):
                    nc.tensor.matmul(
                        pm2,
                        lhsT=hT[:, hb, t2 * 128:(t2 + 1) * 128],
                        rhs=w2_sb[:, hb, :],
                        start=(hb == 0), stop=False,
                    )
                nc.tensor.matmul(
                    pm2, lhsT=ones_col, rhs=b2_sb, start=False, stop=True
                )
                c = tt * 4 + t2
                nc.vector.tensor_add(
                    out=x_sb[:, c, :], in0=x_sb[:, c, :], in1=pm2
                )

    # ---- final LN ----
    lnfw_sb = lnp.tile([128, D], F32, tag="ln1w")
    nc.sync.dma_start(out=lnfw_sb, in_=io["lnfw"][:])
    absorb(lnfw_sb)
    lnfb_sb = lnp.tile([128, D], F32, tag="ln1b")
    nc.sync.dma_start(out=lnfb_sb, in_=io["lnfb"][:])
    absorb(lnfb_sb)

    def final_inplace(c, xn):
        pass

    layer_norm_chunks(lambda c: x_sb[:, c, :], (lnfw_sb, lnfb_sb),
                      final_inplace, inplace_into=lambda c: x_sb[:, c, :])
    nc.sync.dma_start(out=y[:], in_=x_sb)


def _install_ntff_hook():
    """Wire antenv.axon_hooks NTFF profiling via libaxon ctypes (dev only)."""
    if _cache.get("hook_done"):
        return
    _cache["hook_done"] = True
    try:
        import types
        import sys
        try:
            from antenv.axon_hooks import set_axon_ntff_profile_hook  # noqa
        except ImportError:
            import antenv
            mod = types.ModuleType("antenv.axon_hooks")
            holder = [None]
            mod.set_axon_ntff_profile_hook = lambda h: holder.__setitem__(0, h)
            mod.get_axon_ntff_profile_hook = lambda: holder[0]
            sys.modules["antenv.axon_hooks"] = mod
            antenv.axon_hooks = mod
            from trn_agent_boot.trn_boot import _ntff_profile_via_ctypes
            mod.set_axon_ntff_profile_hook(
                _ntff_profile_via_ctypes("/opt/axon/libaxon_pjrt.so"))
    except Exception as e:  # fail-soft: tracing degrades, run still works
        print("ntff hook install failed:", e)


def kernel(tokens, pos_ids, emb_table, input_weight, position_weight,
           ln1_w, ln1_b, ln2_w, ln2_b, w1, b1, w2, b2, lnf_w, lnf_b):
    tokens = np.asarray(tokens)
    pos_ids = np.asarray(pos_ids)
    emb_table = np.asarray(emb_table, dtype=np.float32)
    x0 = (np.float32(np.asarray(input_weight).reshape(-1)[0])
          * emb_table[tokens]
          + np.float32(np.asarray(position_weight).reshape(-1)[0])
          * _pos_table()[np.asarray(pos_ids)][None]).astype(np.float32)

    w1 = np.asarray(w1, np.float32)
    b1 = np.asarray(b1, np.float32)
    w2 = np.asarray(w2, np.float32)
    b2 = np.asarray(b2, np.float32)
    ln2_w = np.asarray(ln2_w, np.float32)
    ln2_b = np.asarray(ln2_b, np.float32)
    # fold LN2 affine into MLP weights
    w1eff = ln2_w[:, :, None] * w1                     # [L, D, H2]
    b1eff = b1 + np.einsum("ld,ldh->lh", ln2_b, w1)    # [L, H2]
    w1h = np.ascontiguousarray(
        w1eff.reshape(L, D // 128, 128, H2).transpose(0, 2, 1, 3))
    w2h = np.ascontiguousarray(
        w2.reshape(L, H2 // 128, 128, D).transpose(0, 2, 1, 3)
    ).astype(ml_dtypes.bfloat16)

    nc = _build_bass()
    base = {
        "biasT": _alibi_biasT(),
        "w1h": w1h,
        "b1h": np.ascontiguousarray(b1eff[:, None, :]),
        "w2h": w2h,
        "b2h": np.ascontiguousarray(np.asarray(b2, np.float32)[:, None, :]),
        "ln1w": np.ascontiguousarray(np.broadcast_to(
            np.asarray(ln1_w, np.float32)[:, None, :], (L, 128, D))),
        "ln1b": np.ascontiguousarray(np.broadcast_to(
            np.asarray(ln1_b, np.float32)[:, None, :], (L, 128, D))),
        "lnfw": np.ascontiguousarray(np.broadcast_to(
            np.asarray(lnf_w, np.float32)[None, :], (128, D))),
        "lnfb": np.ascontiguousarray(np.broadcast_to(
            np.asarray(lnf_b, np.float32)[None, :], (128, D))),
    }
    in_maps = []
    for core in range(NC):
        xc = x0[core * BL:(core + 1) * BL].reshape(T, D)
        xh = np.ascontiguousarray(
            xc.reshape(NCH, 128, D).transpose(1, 0, 2))
        m = dict(base)
        m["x0"] = xh
        in_maps.append(m)

    trace = os.environ.get("KERNEL_TRACE", "0") == "1"
    if trace:
        _install_ntff_hook()
    res = run_bass_kernel_spmd(
        nc, in_maps, core_ids=list(range(NC)), trace=trace,
        trace_cores=[0] if trace else None,
    )
    if trace and res.exec_time_ns is not None:
        print(f"HW exec time: {res.exec_time_ns} ns")
        if res.instructions_and_trace is not None:
            print("trace:", res.instructions_and_trace[1])

    out = np.empty((B, S, D), np.float32)
    for core in range(NC):
        yh = res.results[core]["y"]  # [128, NCH, D]
        yc = yh.transpose(1, 0, 2).reshape(BL, S, D)
        out[core * BL:(core + 1) * BL] = yc
    return out

